# revision 11
# baseline (speedup 1.0000x reference)
"""Trainium2 Bass kernel for nn_AttentionPromptExtrapolation.

Reference computation (B,N,P,D,K = 32,512,25,128,64):
    keep[n,p] = (n not in s_mti) and (p != 24)            # {0,1}, same for all b
    su = sigmoid(patches @ u.T);  su *= (su>0.5) * keep
    sm = sigmoid(patches @ m.T);  sm *= (sm>0.5) * (1-keep)
    out = patches + su @ u + sm @ m

Kernel formulation (halves the matmul work of the reference):
    C = concat(u, m)            [2K=128, D=128]
    z = x @ C.T                 [rows, 128]   (rows = flattened b,n,p)
    z += bias                   bias = -BIG where the (u|m) half is masked
                                (rank-2: outer(bcols, krows) via a K=2 matmul)
    st = (z > 0) * sigmoid(z)   (sigmoid(z)>0.5 <=> z>0; bias kills masked cols)
    out = x + st @ C

Per-core: data-parallel over B, 4 batches per core. The host ships patches
TRANSPOSED ([D, rows] row-major) so the contraction dim D sits on SBUF
partitions with perfectly contiguous 10KB-per-partition DMA chunks — no
on-chip transposes at all. The whole pipeline runs in transposed space
(scores [2K, rows], contribution yT [D, rows], output [D, rows]); the host
un-transposes the result. All matmul stationaries are constants.
The second matmul / add / store stage runs one sub-block behind the score
stage so the PE never stalls waiting on sigmoid/STT.
"""

import numpy as np
import ml_dtypes

import concourse.bass as bass
import concourse.bacc as bacc
import concourse.tile as tile
from concourse import mybir
from concourse.alu_op_type import AluOpType

B, N, P, D, K = 32, 512, 25, 128, 64
K2 = 2 * K              # 128
NCORES = 8
BPC = B // NCORES       # batches per core = 4
NP = N * P              # rows per batch = 12800
BLK = 512               # rows per compute sub-block (one PSUM bank)
NBLK = NP // BLK        # sub-blocks per batch = 25
MB = 5                  # sub-blocks per megablock DMA
MBROWS = MB * BLK       # 2560 rows = 1.25 MB per DMA
T_MTI = 24
BIG = 1e30

F32 = mybir.dt.float32
F16 = mybir.dt.float16
BF16 = mybir.dt.bfloat16


def build_nc(n_blocks=BPC * NBLK):
    """Build the single-core bass program. n_blocks can be reduced for sim
    tests (must be a multiple of MB=5)."""
    assert n_blocks % MB == 0
    n_mb = n_blocks // MB
    rows = n_blocks * BLK
    nc = bacc.Bacc(None, target_bir_lowering=False)

    x_d = nc.dram_tensor("x", [D, rows], F32, kind="ExternalInput")       # x.T
    ct_d = nc.dram_tensor("ct", [D, K2], F32, kind="ExternalInput")       # C.T
    cfp_d = nc.dram_tensor("cfp", [K2, D], F16, kind="ExternalInput")     # C fp16
    keep_d = nc.dram_tensor("keep01", [2, NP], F16, kind="ExternalInput")
    out_d = nc.dram_tensor("out", [D, rows], F32, kind="ExternalOutput")  # out.T

    with tile.TileContext(nc) as tc:
        with (
            tc.tile_pool(name="consts", bufs=1) as consts,
            tc.tile_pool(name="xp", bufs=3) as xp,
            tc.tile_pool(name="sgp", bufs=3) as sgp,
            tc.tile_pool(name="stp", bufs=3) as stp,
            tc.tile_pool(name="op", bufs=2) as op,
            tc.tile_pool(name="ps_z", bufs=3, space="PSUM") as ps_z,
            tc.tile_pool(name="ps_y", bufs=3, space="PSUM") as ps_y,
        ):
            ct_sb = consts.tile([D, K2], F32)
            nc.sync.dma_start(ct_sb, ct_d[:, :])
            cfp_sb = consts.tile([K2, D], F16)
            nc.sync.dma_start(cfp_sb, cfp_d[:, :])
            # keep table broadcast across partitions: rows 0..63 = keep,
            # rows 64..127 = 1-keep (one-time partition-stride-0 DMAs)
            keep_sb = consts.tile([K2, NP], F16)
            for half in range(2):
                src = keep_d[half:half + 1, :]
                bc = bass.AP(
                    tensor=src.tensor, offset=src.offset,
                    ap=[[0, K], src.ap[1]],
                )
                nc.gpsimd.dma_start(out=keep_sb[half * K:(half + 1) * K], in_=bc)

            x_mb = o_mb = None
            pend = None  # previous sub-block's (st_sb, x_mb, o_mb, sub, mb)

            def flush(pend):
                st_sb, px_mb, po_mb, psub, pmb = pend
                # yT [D, rows] = C.T @ st — constant stationary, one matmul
                y_ps = ps_y.tile([128, BLK], F32)
                nc.tensor.matmul(y_ps, lhsT=cfp_sb, rhs=st_sb, start=True, stop=True)
                nc.vector.tensor_tensor(
                    out=po_mb[:, psub * BLK:(psub + 1) * BLK],
                    in0=px_mb[:, psub * BLK:(psub + 1) * BLK],
                    in1=y_ps,
                    op=AluOpType.add,
                )
                if psub == MB - 1:
                    nc.sync.dma_start(
                        out_d[:, pmb * MBROWS:(pmb + 1) * MBROWS], po_mb
                    )

            for i in range(n_blocks):
                mb, sub = divmod(i, MB)
                if sub == 0:
                    x_mb = xp.tile([128, MBROWS], F32)
                    nc.sync.dma_start(x_mb, x_d[:, mb * MBROWS:(mb + 1) * MBROWS])
                    o_mb = op.tile([128, MBROWS], F32)

                # z.T [2K, rows] = C @ x.T
                z_ps = ps_z.tile([128, BLK], F32)
                nc.tensor.matmul(
                    z_ps,
                    lhsT=ct_sb,
                    rhs=x_mb[:, sub * BLK:(sub + 1) * BLK],
                    start=True,
                    stop=True,
                )

                sig_sb = sgp.tile([128, BLK], F16)
                nc.scalar.activation(
                    sig_sb, z_ps, mybir.ActivationFunctionType.Sigmoid
                )
                # mask the (u | m) halves with keep / 1-keep on the idle
                # GpSimd engine (in-place)
                t = i % NBLK
                nc.gpsimd.tensor_tensor(
                    out=sig_sb, in0=sig_sb,
                    in1=keep_sb[:, t * BLK:(t + 1) * BLK],
                    op=AluOpType.mult,
                )
                # st = (z > 0) * sigmoid(z) * keep
                st_sb = stp.tile([128, BLK], F16)
                nc.vector.scalar_tensor_tensor(
                    out=st_sb,
                    in0=z_ps,
                    scalar=0.0,
                    in1=sig_sb,
                    op0=AluOpType.is_gt,
                    op1=AluOpType.mult,
                )

                # second matmul / add / store run one sub-block behind so the
                # PE never waits on the current sub-block's sigmoid/STT
                if pend is not None:
                    flush(pend)
                pend = (st_sb, x_mb, o_mb, sub, mb)

            flush(pend)

    nc.compile()
    return nc


def host_inputs(patches, u_prompt, m_prompt, s_mti):
    """Build the per-core input maps (host-side prep of tables/constants)."""
    patches = np.asarray(patches, dtype=np.float32)
    u = np.asarray(u_prompt, dtype=np.float32)
    m = np.asarray(m_prompt, dtype=np.float32)
    s_mti = np.asarray(s_mti)

    C = np.concatenate([u, m], axis=0)                     # [128, 128]
    ct = np.ascontiguousarray(C.T)                         # [D, 2K] f32
    cfp = np.ascontiguousarray(C.astype(np.float16))

    n_mask = np.ones(N, np.float32)
    n_mask[s_mti] = 0.0
    t_mask = np.ones(P, np.float32)
    t_mask[T_MTI] = 0.0
    keep = (n_mask[:, None] * t_mask[None, :]).reshape(-1)  # [NP]
    keep01 = np.ascontiguousarray(
        np.stack([keep, 1.0 - keep]).astype(np.float16)
    )

    x_flat = patches.reshape(B, NP, D)
    in_maps = []
    for c in range(NCORES):
        xs = np.ascontiguousarray(
            x_flat[c * BPC:(c + 1) * BPC].reshape(BPC * NP, D).T
        )
        in_maps.append({
            "x": xs,
            "ct": ct,
            "cfp": cfp,
            "keep01": keep01,
        })
    return in_maps


_NC_CACHE = {}


def kernel(patches, u_prompt, m_prompt, s_mti, s_uti=None, trace=False, **kw):
    from concourse.bass_utils import run_bass_kernel_spmd

    in_maps = host_inputs(patches, u_prompt, m_prompt, s_mti)

    if "nc" not in _NC_CACHE:
        _NC_CACHE["nc"] = build_nc()
    nc = _NC_CACHE["nc"]

    res = run_bass_kernel_spmd(nc, in_maps, list(range(NCORES)), trace=trace)
    outs = [res.results[c]["out"] for c in range(NCORES)]   # each [D, BPC*NP]
    out = np.concatenate(
        [np.ascontiguousarray(o.T).reshape(BPC, N, P, D) for o in outs], axis=0
    ).astype(np.float32)
    if trace:
        kernel.last_results = res
    return out
